# revision 1
# baseline (speedup 1.0000x reference)
"""RWKV WKV recurrence kernel for Trainium2 (8 NeuronCores).

Math: for each (batch, channel) pair, over time t:
    num_t = a_{t-1} + e^{u+k_t} * v_t
    den_t = b_{t-1} + e^{u+k_t}
    out_t = num_t / den_t
    a_t   = e^w * a_{t-1} + e^{k_t} * v_t
    b_t   = e^w * b_{t-1} + e^{k_t}
with w = -exp(time_decay) < 0, u = time_first. The reference uses a
log-sum-exp-stabilized form of the same recurrence; for these inputs
(k ~ N(0,1), strictly negative decay) the state is geometrically bounded
(|a|,b < ~20) so the direct fp32 form matches to ~1e-7 absmax.

Mapping: batch (8) -> one NeuronCore each. Per core, channels go on SBUF
partitions (16 groups of 128) and time along the free dimension, so the
whole T=2048 recurrence per group is ONE DVE tensor_tensor_scan
(state = ew*state + in, fp32 internal). Inputs arrive [T, H] row-major, so
[128t x 128h] chunks are PE-transposed (via identity matmul) into
channel-major PSUM banks; exp() runs on ScalarE straight out of PSUM.
"""

import os
import sys
from contextlib import ExitStack

import numpy as np

for _p in ("/opt/trn_rl_repo", "/root/.axon_site/_ro/trn_rl_repo"):
    if os.path.isdir(_p) and _p not in sys.path:
        sys.path.insert(0, _p)

import concourse.bacc as bacc
import concourse.mybir as mybir
import concourse.tile as tile
from concourse import masks
from concourse.bass_utils import run_bass_kernel_spmd

F32 = mybir.dt.float32
AF = mybir.ActivationFunctionType
OP = mybir.AluOpType

B, T, H = 8, 2048, 2048
N_CORES = 8


def build_nc(t=T, h=H, recip_mode="accurate"):
    """Build the single-core program (SPMD across cores via differing inputs)."""
    nc = bacc.Bacc("TRN2", target_bir_lowering=False, debug=False)

    key = nc.dram_tensor("key", [t, h], F32, kind="ExternalInput").ap()
    value = nc.dram_tensor("value", [t, h], F32, kind="ExternalInput").ap()
    td = nc.dram_tensor("time_decay", [h], F32, kind="ExternalInput").ap()
    tf = nc.dram_tensor("time_first", [h], F32, kind="ExternalInput").ap()
    out = nc.dram_tensor("out", [t, h], F32, kind="ExternalOutput").ap()

    G = h // 128  # channel groups (partition dim)
    S = t // 128  # 128-wide time chunks
    SB = min(512, t)  # PSUM bank tile free width (512 f32 = 1 bank)
    CPB = SB // 128  # time chunks per PSUM bank tile
    NB = S // CPB  # bank tiles per group

    with tile.TileContext(nc) as tc, ExitStack() as ctx:
        const = ctx.enter_context(tc.tile_pool(name="const", bufs=1))
        identity = const.tile([128, 128], F32)
        masks.make_identity(nc, identity[:])

        tf_t = const.tile([128, G], F32)
        nc.sync.dma_start(tf_t[:], tf.rearrange("(g p) -> p g", p=128))
        td_t = const.tile([128, G], F32)
        nc.sync.dma_start(td_t[:], td.rearrange("(g p) -> p g", p=128))
        eu_t = const.tile([128, G], F32)
        nc.scalar.activation(eu_t[:], tf_t[:], AF.Exp)
        etd_t = const.tile([128, G], F32)
        nc.scalar.activation(etd_t[:], td_t[:], AF.Exp)
        ew_t = const.tile([128, G], F32)  # e^w = exp(-exp(td))
        nc.scalar.activation(ew_t[:], etd_t[:], AF.Exp, scale=-1.0)

        chunks = ctx.enter_context(tc.tile_pool(name="chunks", bufs=6))
        psum_in = ctx.enter_context(tc.tile_pool(name="psum_in", bufs=2, space="PSUM"))
        psum_out = ctx.enter_context(
            tc.tile_pool(name="psum_out", bufs=2, space="PSUM")
        )
        grp = ctx.enter_context(tc.tile_pool(name="grp", bufs=2))
        stage = ctx.enter_context(tc.tile_pool(name="stage", bufs=3))

        for g in range(G):
            eu_g = eu_t[:, g : g + 1]
            ew_g = ew_t[:, g : g + 1]
            hs = slice(g * 128, (g + 1) * 128)

            ek = grp.tile([128, t], F32, tag="ek")
            ekv = grp.tile([128, t], F32, tag="ekv")
            A = grp.tile([128, t + 1], F32, tag="A")
            Bb = grp.tile([128, t + 1], F32, tag="B")
            num = grp.tile([128, t], F32, tag="num")
            den = grp.tile([128, t], F32, tag="den")
            rcp = grp.tile([128, t], F32, tag="rcp")
            outg = grp.tile([128, t], F32, tag="outg")

            # ---- load + transpose to channel-major; exp on ScalarE ----
            for nb in range(NB):
                kT = psum_in.tile([128, SB], F32, tag="kT")
                vT = psum_in.tile([128, SB], F32, tag="vT")
                for c in range(CPB):
                    s = nb * CPB + c
                    ts_ = slice(s * 128, (s + 1) * 128)
                    kc = chunks.tile([128, 128], F32, tag="kc")
                    nc.sync.dma_start(kc[:], key[ts_, hs])
                    vc = chunks.tile([128, 128], F32, tag="vc")
                    nc.sync.dma_start(vc[:], value[ts_, hs])
                    nc.tensor.transpose(
                        kT[:, c * 128 : (c + 1) * 128], kc[:], identity[:]
                    )
                    nc.tensor.transpose(
                        vT[:, c * 128 : (c + 1) * 128], vc[:], identity[:]
                    )
                bsl = slice(nb * SB, (nb + 1) * SB)
                nc.scalar.activation(ek[:, bsl], kT[:], AF.Exp)
                nc.vector.tensor_mul(ekv[:, bsl], ek[:, bsl], vT[:])

            # ---- the recurrence: one scan per group, fp32 state ----
            nc.vector.memset(A[:, 0:1], 0.0)
            nc.vector.memset(Bb[:, 0:1], 0.0)
            d0 = ew_g.broadcast_to((128, t))
            nc.vector.tensor_tensor_scan(
                A[:, 1 : t + 1], d0, ekv[:], 0.0, OP.mult, OP.add
            )
            nc.vector.tensor_tensor_scan(
                Bb[:, 1 : t + 1], d0, ek[:], 0.0, OP.mult, OP.add
            )
            # num = eu*ekv + a_{t-1}; den = eu*ek + b_{t-1}
            nc.vector.scalar_tensor_tensor(
                num[:], ekv[:], eu_g, A[:, 0:t], OP.mult, OP.add
            )
            nc.vector.scalar_tensor_tensor(
                den[:], ek[:], eu_g, Bb[:, 0:t], OP.mult, OP.add
            )
            if recip_mode == "accurate":
                # scratch: ekv is fully consumed by this point
                nc.vector.reciprocal_approx_accurate(rcp[:], den[:], ekv[:])
            elif recip_mode == "fast":
                nc.vector.reciprocal_approx_fast(rcp[:], den[:])
            else:
                nc.vector.reciprocal(rcp[:], den[:])
            nc.vector.tensor_mul(outg[:], num[:], rcp[:])

            # ---- transpose back to [T, H] and store ----
            for nb in range(NB):
                oT = psum_out.tile([128, SB], F32, tag="oT")
                for c in range(CPB):
                    s = nb * CPB + c
                    nc.tensor.transpose(
                        oT[:, c * 128 : (c + 1) * 128],
                        outg[:, s * 128 : (s + 1) * 128],
                        identity[:],
                    )
                ost = stage.tile([128, SB], F32, tag="ost")
                nc.scalar.copy(ost[:], oT[:])
                for c in range(CPB):
                    s = nb * CPB + c
                    nc.sync.dma_start(
                        out[s * 128 : (s + 1) * 128, hs],
                        ost[:, c * 128 : (c + 1) * 128],
                    )

    nc.compile()
    return nc


_nc_cache = {}


def _get_nc():
    if "nc" not in _nc_cache:
        _nc_cache["nc"] = build_nc()
    return _nc_cache["nc"]


def kernel_with_results(key, value, time_decay, time_first, trace=False):
    nc = _get_nc()
    key = np.ascontiguousarray(key, dtype=np.float32)
    value = np.ascontiguousarray(value, dtype=np.float32)
    time_decay = np.ascontiguousarray(time_decay, dtype=np.float32)
    time_first = np.ascontiguousarray(time_first, dtype=np.float32)
    in_maps = [
        {
            "key": key[i],
            "value": value[i],
            "time_decay": time_decay,
            "time_first": time_first,
        }
        for i in range(N_CORES)
    ]
    res = run_bass_kernel_spmd(nc, in_maps, list(range(N_CORES)), trace=trace)
    out = np.stack([res.results[i]["out"] for i in range(N_CORES)], axis=0)
    return out, res


def kernel(key, value, time_decay, time_first):
    out, _ = kernel_with_results(key, value, time_decay, time_first)
    return out
